# revision 24
# baseline (speedup 1.0000x reference)
"""AdderNet CNN forward on 8 TRN2 NeuronCores — pure data parallel over batch.

Reference computation per layer l (8 layers):
  y[b,o,h,w] = -sum_{c,kh,kw} |x[b,c,h+kh-1,w+kw-1] - w[o,c,kh,kw]|   (zero pad)
  x' = relu(s[o]*y + bias[o])
maxpool 2x2 after layers 2, 4, 8; then flatten -> Linear(2048, 10).

Strategy per core (16 images):
  - activations live in SBUF as [channel_partition, (b, Hpad, Wpad)] bf16 with
    zeroed 1-px borders, so conv taps are free-dim offsets
  - im2col: SBUF->SBUF DMA builds compact patch tiles [128 taps, pix]
  - per (o, patch tile): |x-w| = relu(x-w) - min(x-w, 0):
      DVE path: two 4x-mode tensor_scalar ops (add/max, add/min vs -w[o, taps])
      ACT path: one 1x activation(Abs, bias=-w)  (routes a fraction of o's)
  - TensorE reduces over taps: matmul with a +/-1 basis-column lhsT so the
    result lands in psum row o%32 (col group o//32); accumulate over tiles
  - epilogue: one ACT Relu(-s*psum + bias) -> next layer (or pool tmp)
  - FC: 16 accumulated matmuls [128c,10] x [128c,16b] -> psum[10,16] + bias
"""
import numpy as np

B_TOTAL = 128
N_CORES = 8
BC = B_TOTAL // N_CORES  # 16 images per core

# (O, C, Hin, pool_after)
LAYERS = [
    (32, 3, 32, False),
    (32, 32, 32, True),
    (64, 32, 16, False),
    (64, 64, 16, True),
    (128, 64, 8, False),
    (128, 128, 8, False),
    (128, 128, 8, False),
    (128, 128, 8, True),
]

_CACHE = {}


def _layer_geom(li):
    O, C, H, _ = LAYERS[li]
    CKK = C * 9
    T = (CKK + 127) // 128
    n_bch = 4 if H == 32 else (2 if H == 16 else 1)
    npix_c = (BC // n_bch) * H * H
    pch = min(npix_c, 2048)
    return O, CKK, T, pch


def _flavor_counts(T, pch):
    """Per-32-o-group counts (nP, nF, nA) minimizing max engine busy.

    P: PAIRSAD on tile pairs (DVE 1 col/cyc, PE ceil(T/2) streams)
    F: relu tensor_scalar per tile (DVE 4x mode, PE T streams + shared
       ones-stream, |d| = 2 relu(d) - d with sum(x) via ones-matmul and
       sum(w) folded into the bias on host)
    A: ACT Abs per tile (ACT 1 col/cyc, PE T streams)
    """
    pairs = (T + 1) // 2
    # constants fit to CoreSim-measured per-op costs
    p_dve = pairs * (pch / 0.96 + 90)
    f_dve = T * (pch / 4 / 0.96 + 85)
    a_act = T * (pch / 1.2 + 195)
    p_pe = pairs * pch * (1 / 2.4)
    fa_pe = T * pch * (1 / 2.4)
    best = None
    for y in range(33):
        for z in range(33 - y):
            x = 32 - y - z
            dve = x * p_dve + y * f_dve
            act = z * a_act
            pe = x * p_pe + (y + z) * fa_pe + (fa_pe if y else 0.0)
            m = max(dve, act, pe)
            if best is None or m < best[0]:
                best = (m, x, y, z)
    return best[1], best[2], best[3]


def _flavors(li):
    """Static per-o flavor list for layer li ('P'/'F'/'A'), spread evenly
    within each group of 32 (o = q*32 + j, flavor indexed by j)."""
    O, CKK, T, pch = _layer_geom(li)
    if li == 0:
        return ["P"] * O
    nP, nF, nA = _flavor_counts(T, pch)
    flav = []
    cnt = {"P": 0, "F": 0, "A": 0}
    tgt = {"P": nP / 32.0, "F": nF / 32.0, "A": nA / 32.0}
    for j in range(32):
        # pick flavor with count most below target share
        pick = max(("P", "F", "A"),
                   key=lambda k: tgt[k] * (j + 1) - cnt[k])
        cnt[pick] += 1
        flav.append(pick)
    return flav


def _build(cfg=None):
    from contextlib import ExitStack
    import concourse.bacc as bacc
    import concourse.bass as bass
    import concourse.mybir as mybir
    import concourse.tile as tile

    cfg = dict(cfg or {})
    loop_k = cfg.get("loop_k", 0)         # >0: wrap whole net in For_i (timing)

    # custom DVE ops: fused |x0-w0|+|x1-w1| (pair) and |x-w| (single)
    from concourse.dve_spec import Spec, Src0, Src1, C0, C1, maxx, lower, _has_src1
    from concourse.dve_uop import DveOpSpec
    from concourse import dve_ops

    def _register(name, spec):
        for o in dve_ops.OPS:
            if o.name == name:
                return o
        op = dve_ops.DveOp(name, spec, subdim=False, uops_sha={})
        dve_ops.OPS.append(op)
        dve_ops.CUSTOM_DVE_SPECS[name] = spec
        dve_ops._SUB_OPCODE_FOR_NAME[name] = (
            dve_ops._CUSTOM_DVE_ROW_BASE + len(dve_ops.OPS) - 1)
        for ver in ("v3", "v4"):
            s = DveOpSpec(name=name, opcode=dve_ops.get_dve_sub_opcode(name),
                          uops=lower(spec, ver=ver), rd1_en=_has_src1(spec))
            op.uops_sha[ver] = s.sha(ver)
        return op

    PAIRSAD = _register("PAIR_SAD_ANT", Spec(
        body=maxx(Src0 - C0, C0 - Src0) + maxx(Src1 - C1, C1 - Src1),
        reference=lambda in0, in1, s0, s1, imm2: (
            np.abs(in0.astype(np.float32) - np.asarray(s0, np.float32).reshape(-1, 1))
            + np.abs(in1.astype(np.float32) - np.asarray(s1, np.float32).reshape(-1, 1)))))
    ABSD = _register("ABS_DIFF_ANT", Spec(
        body=maxx(Src0 - C0, C0 - Src0),
        reference=lambda in0, in1, s0, s1, imm2: np.abs(
            in0.astype(np.float32) - np.asarray(s0, np.float32).reshape(-1, 1))))

    f32, bf16 = mybir.dt.float32, mybir.dt.bfloat16
    A = mybir.AluOpType
    AF = mybir.ActivationFunctionType

    nc = bacc.Bacc("TRN2", target_bir_lowering=False, debug=False)

    x_d = nc.dram_tensor("x", [BC, 3, 32, 32], f32, kind="ExternalInput")
    w_d, s_d, b_d = {}, {}, {}
    for i, (O, C, H, _) in enumerate(LAYERS):
        w_d[i] = nc.dram_tensor(f"w{i+1}", [O, C, 3, 3], f32, kind="ExternalInput")
        s_d[i] = nc.dram_tensor(f"s{i+1}", [O], f32, kind="ExternalInput")
        # host-adjusted bias: b - s*sum(w) on F-flavored o rows
        b_d[i] = nc.dram_tensor(f"bbx{i+1}", [O], f32, kind="ExternalInput")
    fcw_d = nc.dram_tensor("fc_w", [10, 2048], f32, kind="ExternalInput")
    fcb_d = nc.dram_tensor("fc_b", [10], f32, kind="ExternalInput")
    out_d = nc.dram_tensor("out", [BC, 10], f32, kind="ExternalOutput")

    with tile.TileContext(nc) as tc, ExitStack() as ctx:
        persist = ctx.enter_context(tc.tile_pool(name="persist", bufs=1))
        wpool = ctx.enter_context(tc.tile_pool(name="wpool", bufs=1))
        dpool = ctx.enter_context(tc.tile_pool(name="dpool", bufs=5))
        pspool = ctx.enter_context(tc.tile_pool(name="pspool", bufs=2, space="PSUM"))

        # padded activation tensors, channel-partition, (b, H+2, W+2) free
        Ap = []  # entry i: input to layer i
        shapes = []
        for i, (O, C, H, _) in enumerate(LAYERS):
            shapes.append((C, H))
        for i, (C, H) in enumerate(shapes):
            if i == 0:
                # image-major [b*3+c, (H+2)*(W+2)]: avoids reserving a
                # BC*34*34 free range on a 3-partition tensor
                t = persist.tile([3 * BC, (H + 2) * (H + 2)], bf16,
                                 name=f"Ap{i}", tag=f"Ap{i}")
            else:
                t = persist.tile([C, BC * (H + 2) * (H + 2)], bf16,
                                 name=f"Ap{i}", tag=f"Ap{i}")
            nc.vector.memset(t, 0.0)
            Ap.append(t)
        A8 = persist.tile([128, BC * 16], bf16, name="A8", tag="A8")  # FC input

        # basis tensors: Tpos/Tneg [128, 64], column 32 = +/-1; Tpos2 = +2
        Tpos = persist.tile([128, 64], bf16, name="Tpos", tag="Tpos")
        Tneg = persist.tile([128, 64], bf16, name="Tneg", tag="Tneg")
        Tpos2 = persist.tile([128, 64], bf16, name="Tpos2", tag="Tpos2")
        nc.vector.memset(Tpos, 0.0)
        nc.vector.memset(Tneg, 0.0)
        nc.vector.memset(Tpos2, 0.0)
        nc.vector.memset(Tpos[:, 32:33], 1.0)
        nc.vector.memset(Tneg[:, 32:33], -1.0)
        nc.vector.memset(Tpos2[:, 32:33], 2.0)

        # per-(layer, group) ones-mask lhsT [128, 32]: col j = -1 if o is F
        ones_mask = {}
        for li in range(1, 8):
            O_l = LAYERS[li][0]
            flav = _flavors(li)
            for q in range(max(1, O_l // 32)):
                nQ_l = max(1, O_l // 32)
                cols = [j for j in range(32)
                        if q * 32 + j < O_l
                        and flav[(j * nQ_l + q) % 32] == "F"]
                if not cols:
                    continue
                m = persist.tile([128, 32], bf16, name=f"om{li}_{q}",
                                 tag=f"om{li}_{q}")
                nc.vector.memset(m, 0.0)
                for j in cols:
                    nc.vector.memset(m[:, j:j + 1], -1.0)
                ones_mask[(li, q)] = m

        # load input x -> Ap[0] interior (f32 -> bf16); rows = (b, c)
        a0v = Ap[0].rearrange("p (h w) -> p h w", h=34)
        with tc.tile_pool(name="xload", bufs=1) as xpool:
            xs = xpool.tile([48, 1024], f32, name="xs", tag="xs")
            nc.sync.dma_start(out=xs, in_=bass.AP(
                tensor=x_d, offset=0, ap=[[1024, 48], [1, 1024]]))
            nc.vector.tensor_copy(
                a0v[0:48, 1:33, 1:33],
                xs.rearrange("p (h w) -> p h w", h=32))

        # per-layer weights, f = blk*C + c:
        #   wpos[t] [rows, O] f32 = +w (custom-DVE ops), wneg = -w (ACT bias)
        wpos_all, wneg_all, negs_all, bb_all = [], [], [], []
        for i, (O, C, H, _) in enumerate(LAYERS):
            CKK = C * 9
            T = (CKK + 127) // 128
            wpos_l, wneg_l = [], []
            for t in range(T):
                rows = min(128, CKK - t * 128)
                wps = wpool.tile([rows, O], f32, name=f"wps{i}_{t}",
                                 tag="wstage", bufs=2)
                blk0 = t * 128 // C
                nblk = rows // C
                for bi in range(nblk):
                    blk = blk0 + bi
                    nc.sync.dma_start(
                        out=wps[bi * C:(bi + 1) * C, :],
                        in_=bass.AP(tensor=w_d[i], offset=blk,
                                    ap=[[9, C], [C * 9, O]]))
                wp = wpool.tile([rows, O], f32, name=f"wpos{i}_{t}",
                                tag=f"wpos{i}_{t}")
                nc.vector.tensor_copy(wp, wps)
                wn = wpool.tile([rows, O], f32, name=f"wneg{i}_{t}",
                                tag=f"wneg{i}_{t}")
                nc.vector.tensor_scalar_mul(wn, wps, -1.0)
                wpos_l.append(wp)
                wneg_l.append(wn)
            wpos_all.append(wpos_l)
            wneg_all.append(wneg_l)

            st = wpool.tile([O, 1], f32, name=f"st{i}", tag="st_tmp", bufs=2)
            nc.sync.dma_start(out=st, in_=bass.AP(tensor=s_d[i], offset=0,
                                                  ap=[[1, O], [1, 1]]))
            ns = wpool.tile([O, 1], f32, name=f"negs{i}", tag=f"negs{i}")
            nc.vector.tensor_scalar_mul(ns, st, -1.0)
            negs_all.append(ns)
            bb = wpool.tile([O, 1], f32, name=f"bb{i}", tag=f"bb{i}")
            nc.sync.dma_start(out=bb, in_=bass.AP(tensor=b_d[i], offset=0,
                                                  ap=[[1, O], [1, 1]]))
            bb_all.append(bb)

        # FC weights [c, (hw, cls)] bf16 and bias [10, 1] f32
        fcs = persist.tile([128, 160], f32, name="fcs", tag="fcs")
        nc.sync.dma_start(out=fcs, in_=bass.AP(
            tensor=fcw_d, offset=0, ap=[[16, 128], [1, 16], [2048, 10]]))
        fcw = persist.tile([128, 160], bf16, name="fcw", tag="fcw")
        nc.vector.tensor_copy(fcw, fcs)
        fcb = persist.tile([10, 1], f32, name="fcb", tag="fcb")
        nc.sync.dma_start(out=fcb, in_=bass.AP(tensor=fcb_d, offset=0,
                                               ap=[[1, 10], [1, 1]]))

        # --- L1 replicated-path constants ---
        # wneg1x4 [108, 32]: rows 27r+f = -w1[o, f] (4 replicas)
        wpos4 = persist.tile([108, 32], f32, name="wpos4", tag="wpos4")
        for r in range(4):
            for blk in range(9):
                nc.sync.dma_start(
                    out=wpos4[27 * r + 3 * blk:27 * r + 3 * blk + 3, :],
                    in_=bass.AP(tensor=w_d[0], offset=blk, ap=[[9, 3], [27, 32]]))
        wneg1x4 = persist.tile([108, 32], f32, name="wneg1x4", tag="wneg1x4")
        nc.vector.tensor_scalar_mul(wneg1x4, wpos4, -1.0)
        # R2p/R2n [108, 64] bf16: col 32+8r is +/-1 on replica-r rows.
        # Build via a [1, 256] onehot strip (partition-0 writes are aligned),
        # then broadcast-DMA each 64-segment to the replica's 27 rows.
        strip = persist.tile([1, 512], bf16, name="strip", tag="strip")
        nc.vector.memset(strip, 0.0)
        for r in range(4):
            nc.vector.memset(strip[0:1, 64 * r + 32 + 8 * r:64 * r + 33 + 8 * r], 1.0)
            nc.vector.memset(strip[0:1, 256 + 64 * r + 32 + 8 * r:
                                  256 + 64 * r + 33 + 8 * r], -1.0)
        strip_d = nc.dram_tensor("r2strip", [512], bf16)
        nc.sync.dma_start(out=bass.AP(tensor=strip_d, offset=0,
                                      ap=[[512, 1], [1, 512]]),
                          in_=strip[0:1, :])
        R2p = persist.tile([108, 64], bf16, name="R2p", tag="R2p")
        R2n = persist.tile([108, 64], bf16, name="R2n", tag="R2n")
        for r in range(4):
            nc.sync.dma_start(out=R2p[27 * r:27 * r + 27, :], in_=bass.AP(
                tensor=strip_d, offset=64 * r, ap=[[0, 27], [1, 64]]))
            nc.sync.dma_start(out=R2n[27 * r:27 * r + 27, :], in_=bass.AP(
                tensor=strip_d, offset=256 + 64 * r, ap=[[0, 27], [1, 64]]))
        # negs1e/bb1e per o-group g: [32,1], row 8r+c = value[8g+c]
        negs1e, bb1e = [], []
        for g in range(4):
            se = wpool.tile([32, 1], f32, name=f"se1_{g}", tag=f"se1_{g}")
            be = wpool.tile([32, 1], f32, name=f"be1_{g}", tag=f"be1_{g}")
            for r in range(4):
                nc.sync.dma_start(out=se[8 * r:8 * r + 8, :], in_=bass.AP(
                    tensor=s_d[0], offset=8 * g, ap=[[1, 8], [1, 1]]))
                nc.sync.dma_start(out=be[8 * r:8 * r + 8, :], in_=bass.AP(
                    tensor=b_d[0], offset=8 * g, ap=[[1, 8], [1, 1]]))
            ne = wpool.tile([32, 1], f32, name=f"ne1_{g}", tag=f"ne1_{g}")
            nc.vector.tensor_scalar_mul(ne, se, -1.0)
            negs1e.append(ne)
            bb1e.append(be)

        def l1_body():
            # layer 1, replicated: P1 [108, 1024] rows 27r+f, image b0+r
            O, C, H = 32, 3, 32
            W = H
            srcv = Ap[0].rearrange("p (h w) -> p h w", h=H + 2)
            dstv = Ap[1].rearrange("c (b h w) -> c b h w", b=BC, h=H + 2)
            with ExitStack() as lctx:
                p1pool = lctx.enter_context(tc.tile_pool(name="patchL1", bufs=2))
                for bch in range(4):
                    b0 = bch * 4
                    P1 = p1pool.tile([108, 1024], bf16, name=f"P1_{bch}", tag="P1")
                    for r in range(4):
                        for blk in range(9):
                            dh, dw = blk // 3, blk % 3
                            nc.sync.dma_start(
                                out=P1[27 * r + 3 * blk:27 * r + 3 * blk + 3,
                                       :].rearrange("c (h w) -> c h w", h=H),
                                in_=srcv[3 * (b0 + r):3 * (b0 + r) + 3,
                                         dh:dh + H, dw:dw + W])
                    for g in range(4):
                        ps = pspool.tile([32, 1024], f32, name=f"psL1_{bch}_{g}",
                                         tag="ps")
                        for ol in range(8):
                            o = 8 * g + ol
                            r1 = dpool.tile([108, 1024], bf16, name="r1L1", tag="d")
                            r2 = dpool.tile([108, 1024], bf16, name="r2L1", tag="d")
                            nc.vector.tensor_scalar(
                                r1, P1, wneg1x4[:, o:o + 1], 0.0, A.add, A.max)
                            nc.vector.tensor_scalar(
                                r2, P1, wneg1x4[:, o:o + 1], 0.0, A.add, A.min)
                            for sl in range(2):
                                nc.tensor.matmul(
                                    ps[0:32, sl * 512:(sl + 1) * 512],
                                    R2p[:, 32 - ol:64 - ol],
                                    r1[:, sl * 512:(sl + 1) * 512],
                                    start=(ol == 0), stop=False,
                                    tile_position=(0, 0), skip_group_check=True)
                                nc.tensor.matmul(
                                    ps[0:32, sl * 512:(sl + 1) * 512],
                                    R2n[:, 32 - ol:64 - ol],
                                    r2[:, sl * 512:(sl + 1) * 512],
                                    start=False, stop=(ol == 7),
                                    tile_position=(0, 0), skip_group_check=True)
                        tmpL1 = dpool.tile([32, 1024], bf16, name="tmpL1",
                                           tag="tmpl1", bufs=2)
                        nc.scalar.activation(tmpL1, ps, AF.Relu,
                                             bias=bb1e[g], scale=negs1e[g])
                        for r in range(4):
                            nc.sync.dma_start(
                                out=dstv[8 * g:8 * g + 8, b0 + r, 1:H + 1, 1:W + 1],
                                in_=tmpL1[8 * r:8 * r + 8, :].rearrange(
                                    "c (h w) -> c h w", h=H))

        def net_body():
            l1_body()
            for li, (O, C, H, pool_after) in enumerate(LAYERS):
                if li == 0:
                    continue
                CKK = C * 9
                T = (CKK + 127) // 128
                Hp = H + 2
                W = H
                src = Ap[li]
                srcv = src.rearrange("c (b h w) -> c b h w", b=BC, h=Hp)
                nQ = max(1, O // 32)
                # batch chunking: big layers processed in halves
                n_bch = 4 if H == 32 else (2 if H == 16 else 1)
                bcs = BC // n_bch
                npix_c = bcs * H * W
                # psum pix chunk (4 banks/tile x 2 bufs = all 8 banks)
                pch = min(npix_c, 2048)

                with ExitStack() as lctx:
                    ppool = lctx.enter_context(
                        tc.tile_pool(name=f"patch{li}", bufs=1))
                    tpool = (lctx.enter_context(
                        tc.tile_pool(name=f"ptmp{li}", bufs=1))
                        if pool_after else None)

                    for bch in range(n_bch):
                        b0 = bch * bcs
                        # --- build patch tiles via SBUF->SBUF DMA ---
                        pt = []
                        for t in range(T):
                            rows = min(128, CKK - t * 128)
                            p = ppool.tile([rows, npix_c], bf16,
                                           name=f"p{li}_{bch}_{t}", tag=f"pt{t}")
                            pt.append(p)
                        for blk in range(9):
                            dh, dw = blk // 3, blk % 3
                            gr = blk * C
                            t, r0 = gr // 128, gr % 128
                            if H == 8 and r0 % 32 == 0 and C % 32 == 0:
                                nc.gpsimd.tensor_copy(
                                    pt[t][r0:r0 + C, :].rearrange(
                                        "c (b h w) -> c b h w", b=bcs, h=H),
                                    srcv[0:C, b0:b0 + bcs, dh:dh + H, dw:dw + W])
                            else:
                                for bi in range(bcs):
                                    nc.sync.dma_start(
                                        out=pt[t][r0:r0 + C,
                                                  bi * H * W:(bi + 1) * H * W].rearrange(
                                            "c (h w) -> c h w", h=H),
                                        in_=srcv[0:C, b0 + bi, dh:dh + H, dw:dw + W])

                        if pool_after:
                            dest = tpool.tile([O, npix_c], bf16,
                                              name=f"tmp{li}_{bch}", tag="tmp")
                        # --- absdiff + PE reduce + epilogue, per psum chunk ---
                        flav = _flavors(li)
                        npair = T // 2
                        for p0 in range(0, npix_c, pch):
                            ps = pspool.tile([max(32, O), pch], f32,
                                             name=f"ps{li}_{bch}_{p0}", tag="ps")
                            nsl = pch // 512
            # ones-streams are emitted AFTER the o-streams (inside the
                            # same psum accumulation group) so the first PE
                            # work of a chunk is not gated on every patch DMA
                            for j in range(32):
                                for q in range(nQ):
                                    o = q * 32 + j
                                    if o >= O:
                                        continue
                                    fl = flav[(j * nQ + q) % 32]
                                    streams = []
                                    if fl == "A":
                                        for t in range(T):
                                            rows = pt[t].shape[0]
                                            d = dpool.tile([rows, pch], bf16,
                                                           name=f"d{li}",
                                                           tag="dA", bufs=8)
                                            nc.scalar.activation(
                                                d, pt[t][:, p0:p0 + pch], AF.Abs,
                                                bias=wneg_all[li][t][:, o:o + 1],
                                                scale=1.0)
                                            streams.append((rows, d, Tpos))
                                    elif fl == "F":
                                        for t in range(T):
                                            rows = pt[t].shape[0]
                                            d = dpool.tile([rows, pch], bf16,
                                                           name=f"df{li}", tag="d")
                                            nc.vector.tensor_scalar(
                                                d, pt[t][:, p0:p0 + pch],
                                                wneg_all[li][t][:, o:o + 1],
                                                0.0, A.add, A.max)
                                            streams.append((rows, d, Tpos2))
                                    else:
                                        for pi in range(npair):
                                            t0, t1 = 2 * pi, 2 * pi + 1
                                            rows = pt[t0].shape[0]
                                            d = dpool.tile([rows, pch], bf16,
                                                           name=f"dp{li}", tag="d")
                                            nc.vector._custom_dve(
                                                PAIRSAD, out=d[:, :],
                                                in0=pt[t0][:, p0:p0 + pch],
                                                in1=pt[t1][:, p0:p0 + pch],
                                                s0=wpos_all[li][t0][:, o:o + 1],
                                                s1=wpos_all[li][t1][:, o:o + 1])
                                            streams.append((rows, d, Tpos))
                                        if T % 2:
                                            t0 = T - 1
                                            rows = pt[t0].shape[0]
                                            d = dpool.tile([rows, pch], bf16,
                                                           name=f"ds{li}", tag="d")
                                            nc.vector._custom_dve(
                                                ABSD, out=d[:, :],
                                                in0=pt[t0][:, p0:p0 + pch],
                                                s0=wpos_all[li][t0][:, o:o + 1])
                                            streams.append((rows, d, Tpos))
                                    nstr = len(streams)
                                    last_o = ((j == 31) or (o == O - 1))
                                    for si, (rows, d, basis) in enumerate(streams):
                                        for sl in range(nsl):
                                            nc.tensor.matmul(
                                                ps[q * 32:q * 32 + 32,
                                                   sl * 512:(sl + 1) * 512],
                                                basis[0:rows, 32 - j:64 - j],
                                                d[:, sl * 512:(sl + 1) * 512],
                                                start=(j == 0 and si == 0),
                                                stop=(last_o and si == nstr - 1
                                                      and (li, q) not in ones_mask),
                                                tile_position=(0, 32 * q),
                                                skip_group_check=True)
                            # trailing ones-streams: add -sum(x) into F rows
                            for q in range(nQ):
                                om = ones_mask.get((li, q))
                                if om is None:
                                    continue
                                for t in range(T):
                                    rows = pt[t].shape[0]
                                    for sl in range(nsl):
                                        nc.tensor.matmul(
                                            ps[q * 32:q * 32 + 32,
                                               sl * 512:(sl + 1) * 512],
                                            om[0:rows, :],
                                            pt[t][:, p0 + sl * 512:
                                                  p0 + (sl + 1) * 512],
                                            start=False,
                                            stop=(t == T - 1 and sl == nsl - 1),
                                            tile_position=(0, 32 * q),
                                            skip_group_check=True)
                            # epilogue: relu(-s * psum + b)
                            if pool_after:
                                nc.scalar.activation(
                                    dest[:, p0:p0 + pch], ps[0:O, :], AF.Relu,
                                    bias=bb_all[li][:, :], scale=negs_all[li][:, :])
                            else:
                                Hn = H  # same spatial size, next layer pad Hn+2
                                dv = Ap[li + 1].rearrange(
                                    "c (b h w) -> c b h w", b=BC, h=Hn + 2)
                                # pixel range [p0, p0+pch) within this bchunk:
                                # whole images per chunk (pch % (H*W) == 0)
                                i0 = b0 + p0 // (H * W)
                                ni = pch // (H * W)
                                nc.scalar.activation(
                                    dv[0:O, i0:i0 + ni, 1:H + 1, 1:W + 1],
                                    ps[0:O, :].rearrange(
                                        "c (b h w) -> c b h w", b=ni, h=H),
                                    AF.Relu,
                                    bias=bb_all[li][:, :], scale=negs_all[li][:, :])

                        # --- maxpool 2x2 -> next padded tensor (or A8) ---
                        if pool_after:
                            dv4 = dest.rearrange("c (b h w) -> c b h w", b=bcs, h=H)
                            m1 = tpool.tile([O, npix_c // 2], bf16,
                                            name=f"m1_{li}_{bch}", tag="m1")
                            m1v = m1.rearrange("c (b h w) -> c b h w", b=bcs, h=H)
                            nc.vector.tensor_tensor(
                                m1v, dv4[:, :, :, 0::2], dv4[:, :, :, 1::2], A.max)
                            if li == 7:
                                nxt = A8.rearrange("c (b h w) -> c b h w",
                                                   b=BC, h=4)[0:O, b0:b0 + bcs]
                            else:
                                Hn = H // 2
                                nxt = Ap[li + 1].rearrange(
                                    "c (b h w) -> c b h w", b=BC, h=Hn + 2)[
                                    0:O, b0:b0 + bcs, 1:Hn + 1, 1:Hn + 1]
                            nc.vector.tensor_tensor(
                                nxt, m1v[:, :, 0::2, :], m1v[:, :, 1::2, :], A.max)

            # --- FC: out[cls, b] = sum_c,hw A8[c, b*16+hw] * fcw[c, hw*10+cls] ---
            psf = pspool.tile([32, 512], f32, name="psf", tag="ps")
            for hw in range(16):
                nc.tensor.matmul(
                    psf[0:10, 0:BC],
                    fcw[:, hw * 10:(hw + 1) * 10],
                    bass.AP(tensor=A8.tensor, offset=A8.offset + hw,
                            ap=[list(A8.ap[0]), [16, BC]]),
                    start=(hw == 0), stop=(hw == 15), skip_group_check=True)
            osb = persist.tile([10, BC], f32, name="osb", tag="osb")
            nc.scalar.activation(osb, psf[0:10, 0:BC], AF.Identity,
                                 bias=fcb[:, :], scale=1.0)
            nc.sync.dma_start(
                out=bass.AP(tensor=out_d, offset=0, ap=[[1, 10], [10, BC]]),
                in_=osb)

        if loop_k > 1:
            with tc.For_i(0, loop_k, 1):
                net_body()
        else:
            net_body()

    nc.compile()
    return nc


def _get_nc(cfg=None):
    key = str(sorted((cfg or {}).items()))
    if key not in _CACHE:
        _CACHE[key] = _build(cfg)
    return _CACHE[key]


def make_in_maps(inputs):
    full = dict(inputs)
    x = np.ascontiguousarray(full["x"], dtype=np.float32)
    reps = {}
    for i in range(1, 9):
        w = np.ascontiguousarray(full[f"w{i}"], np.float32)
        s = np.ascontiguousarray(full[f"s{i}"], np.float32)
        b = np.ascontiguousarray(full[f"b{i}"], np.float32)
        O = w.shape[0]
        flav = _flavors(i - 1)
        nQ_l = max(1, O // 32)
        is_f = np.array(
            [flav[((o % 32) * nQ_l + o // 32) % 32] == "F" for o in range(O)],
            np.float32)
        wsum = w.reshape(O, -1).sum(axis=1)
        reps[f"w{i}"] = w
        reps[f"s{i}"] = s
        reps[f"bbx{i}"] = (b - s * wsum * is_f).astype(np.float32)
    reps["fc_w"] = np.ascontiguousarray(full["fc_w"], np.float32)
    reps["fc_b"] = np.ascontiguousarray(full["fc_b"], np.float32)
    in_maps = []
    for c in range(N_CORES):
        m = {"x": x[c * BC:(c + 1) * BC]}
        m.update(reps)
        in_maps.append(m)
    return in_maps


def kernel(**inputs):
    from concourse import bass_utils
    nc = _get_nc()
    in_maps = make_in_maps(inputs)
    res = bass_utils.run_bass_kernel_spmd(nc, in_maps,
                                          core_ids=list(range(N_CORES)))
    return np.concatenate([r["out"] for r in res.results], axis=0)



# revision 26
# speedup vs baseline: 1.0461x; 1.0461x over previous
"""AdderNet CNN forward on 8 TRN2 NeuronCores — pure data parallel over batch.

Reference computation per layer l (8 layers):
  y[b,o,h,w] = -sum_{c,kh,kw} |x[b,c,h+kh-1,w+kw-1] - w[o,c,kh,kw]|   (zero pad)
  x' = relu(s[o]*y + bias[o])
maxpool 2x2 after layers 2, 4, 8; then flatten -> Linear(2048, 10).

Strategy per core (16 images):
  - activations live in SBUF as [channel_partition, (b, Hpad, Wpad)] bf16 with
    zeroed 1-px borders, so conv taps are free-dim offsets
  - im2col: SBUF->SBUF DMA builds compact patch tiles [128 taps, pix]
  - per (o, patch tile), three flavors balanced per layer across engines
    (counts from _flavor_counts, rotated across psum groups so the PE
    stream alternates DVE- and ACT-produced tiles):
      P: custom-DVE PAIRSAD |x-w0|+|x-w1| on tile pairs (1 col/cyc, but
         halves PE streams)
      F: one 4x-mode tensor_scalar relu(x-w); |d| = 2 relu(d) - d, with
         sum_f x from a shared trailing ones-matmul into F rows (-1 mask
         lhsT) and sum_f w folded into the bias on the host (bbx inputs)
      A: ACT activation(Abs, bias=-w), own deeper buffer ring (tag dA)
  - TensorE reduces over taps: matmul with a +1/+2 basis-column lhsT so the
    result lands in psum row o%32 (col group o//32); accumulate over tiles
  - epilogue: one ACT Relu(-s*psum + bbx) -> next layer (or pool tmp)
  - FC: 16 accumulated matmuls [128c,10] x [128c,16b] -> psum[10,16] + bias
"""
import numpy as np

B_TOTAL = 128
N_CORES = 8
BC = B_TOTAL // N_CORES  # 16 images per core

# (O, C, Hin, pool_after)
LAYERS = [
    (32, 3, 32, False),
    (32, 32, 32, True),
    (64, 32, 16, False),
    (64, 64, 16, True),
    (128, 64, 8, False),
    (128, 128, 8, False),
    (128, 128, 8, False),
    (128, 128, 8, True),
]

_CACHE = {}


def _layer_geom(li):
    O, C, H, _ = LAYERS[li]
    CKK = C * 9
    T = (CKK + 127) // 128
    n_bch = 4 if H == 32 else (2 if H == 16 else 1)
    npix_c = (BC // n_bch) * H * H
    pch = min(npix_c, 2048)
    return O, CKK, T, pch


def _flavor_counts(T, pch):
    """Per-32-o-group counts (nP, nF, nA) minimizing max engine busy.

    P: PAIRSAD on tile pairs (DVE 1 col/cyc, PE ceil(T/2) streams)
    F: relu tensor_scalar per tile (DVE 4x mode, PE T streams + shared
       ones-stream, |d| = 2 relu(d) - d with sum(x) via ones-matmul and
       sum(w) folded into the bias on host)
    A: ACT Abs per tile (ACT 1 col/cyc, PE T streams)
    """
    pairs = (T + 1) // 2
    # constants fit to CoreSim-measured per-op costs
    p_dve = pairs * (pch / 0.96 + 90)
    f_dve = T * (pch / 4 / 0.96 + 85)
    a_act = T * (pch / 1.2 + 195)
    p_pe = pairs * pch * (1 / 2.4)
    fa_pe = T * pch * (1 / 2.4)
    best = None
    for y in range(33):
        for z in range(33 - y):
            x = 32 - y - z
            dve = x * p_dve + y * f_dve
            act = z * a_act
            pe = x * p_pe + (y + z) * fa_pe + (fa_pe if y else 0.0)
            m = max(dve, act, pe)
            if best is None or m < best[0]:
                best = (m, x, y, z)
    return best[1], best[2], best[3]


def _flavors(li):
    """Static per-o flavor list for layer li ('P'/'F'/'A'), spread evenly
    within each group of 32 (o = q*32 + j, flavor indexed by j)."""
    O, CKK, T, pch = _layer_geom(li)
    if li == 0:
        return ["P"] * O
    nP, nF, nA = _flavor_counts(T, pch)
    flav = []
    cnt = {"P": 0, "F": 0, "A": 0}
    tgt = {"P": nP / 32.0, "F": nF / 32.0, "A": nA / 32.0}
    for j in range(32):
        # pick flavor with count most below target share
        pick = max(("P", "F", "A"),
                   key=lambda k: tgt[k] * (j + 1) - cnt[k])
        cnt[pick] += 1
        flav.append(pick)
    return flav


def _build(cfg=None):
    from contextlib import ExitStack
    import concourse.bacc as bacc
    import concourse.bass as bass
    import concourse.mybir as mybir
    import concourse.tile as tile

    cfg = dict(cfg or {})
    loop_k = cfg.get("loop_k", 0)         # >0: wrap whole net in For_i (timing)

    # custom DVE ops: fused |x0-w0|+|x1-w1| (pair) and |x-w| (single)
    from concourse.dve_spec import Spec, Src0, Src1, C0, C1, maxx, lower, _has_src1
    from concourse.dve_uop import DveOpSpec
    from concourse import dve_ops

    def _register(name, spec):
        for o in dve_ops.OPS:
            if o.name == name:
                return o
        op = dve_ops.DveOp(name, spec, subdim=False, uops_sha={})
        dve_ops.OPS.append(op)
        dve_ops.CUSTOM_DVE_SPECS[name] = spec
        dve_ops._SUB_OPCODE_FOR_NAME[name] = (
            dve_ops._CUSTOM_DVE_ROW_BASE + len(dve_ops.OPS) - 1)
        for ver in ("v3", "v4"):
            s = DveOpSpec(name=name, opcode=dve_ops.get_dve_sub_opcode(name),
                          uops=lower(spec, ver=ver), rd1_en=_has_src1(spec))
            op.uops_sha[ver] = s.sha(ver)
        return op

    PAIRSAD = _register("PAIR_SAD_ANT", Spec(
        body=maxx(Src0 - C0, C0 - Src0) + maxx(Src1 - C1, C1 - Src1),
        reference=lambda in0, in1, s0, s1, imm2: (
            np.abs(in0.astype(np.float32) - np.asarray(s0, np.float32).reshape(-1, 1))
            + np.abs(in1.astype(np.float32) - np.asarray(s1, np.float32).reshape(-1, 1)))))
    ABSD = _register("ABS_DIFF_ANT", Spec(
        body=maxx(Src0 - C0, C0 - Src0),
        reference=lambda in0, in1, s0, s1, imm2: np.abs(
            in0.astype(np.float32) - np.asarray(s0, np.float32).reshape(-1, 1))))

    f32, bf16 = mybir.dt.float32, mybir.dt.bfloat16
    A = mybir.AluOpType
    AF = mybir.ActivationFunctionType

    nc = bacc.Bacc("TRN2", target_bir_lowering=False, debug=False)

    x_d = nc.dram_tensor("x", [BC, 3, 32, 32], f32, kind="ExternalInput")
    w_d, s_d, b_d = {}, {}, {}
    for i, (O, C, H, _) in enumerate(LAYERS):
        w_d[i] = nc.dram_tensor(f"w{i+1}", [O, C, 3, 3], f32, kind="ExternalInput")
        s_d[i] = nc.dram_tensor(f"s{i+1}", [O], f32, kind="ExternalInput")
        # host-adjusted bias: b - s*sum(w) on F-flavored o rows
        b_d[i] = nc.dram_tensor(f"bbx{i+1}", [O], f32, kind="ExternalInput")
    fcw_d = nc.dram_tensor("fc_w", [10, 2048], f32, kind="ExternalInput")
    fcb_d = nc.dram_tensor("fc_b", [10], f32, kind="ExternalInput")
    out_d = nc.dram_tensor("out", [BC, 10], f32, kind="ExternalOutput")

    with tile.TileContext(nc) as tc, ExitStack() as ctx:
        persist = ctx.enter_context(tc.tile_pool(name="persist", bufs=1))
        wpool = ctx.enter_context(tc.tile_pool(name="wpool", bufs=1))
        dpool = ctx.enter_context(tc.tile_pool(name="dpool", bufs=6))
        pspool = ctx.enter_context(tc.tile_pool(name="pspool", bufs=2, space="PSUM"))

        # padded activation tensors, channel-partition, (b, H+2, W+2) free
        Ap = []  # entry i: input to layer i
        shapes = []
        for i, (O, C, H, _) in enumerate(LAYERS):
            shapes.append((C, H))
        for i, (C, H) in enumerate(shapes):
            if i == 0:
                # image-major [b*3+c, (H+2)*(W+2)]: avoids reserving a
                # BC*34*34 free range on a 3-partition tensor
                t = persist.tile([3 * BC, (H + 2) * (H + 2)], bf16,
                                 name=f"Ap{i}", tag=f"Ap{i}")
            else:
                t = persist.tile([C, BC * (H + 2) * (H + 2)], bf16,
                                 name=f"Ap{i}", tag=f"Ap{i}")
            nc.vector.memset(t, 0.0)
            Ap.append(t)
        A8 = persist.tile([128, BC * 16], bf16, name="A8", tag="A8")  # FC input

        # basis tensors: Tpos/Tneg [128, 64], column 32 = +/-1; Tpos2 = +2
        Tpos = persist.tile([128, 64], bf16, name="Tpos", tag="Tpos")
        Tneg = persist.tile([128, 64], bf16, name="Tneg", tag="Tneg")
        Tpos2 = persist.tile([128, 64], bf16, name="Tpos2", tag="Tpos2")
        nc.vector.memset(Tpos, 0.0)
        nc.vector.memset(Tneg, 0.0)
        nc.vector.memset(Tpos2, 0.0)
        nc.vector.memset(Tpos[:, 32:33], 1.0)
        nc.vector.memset(Tneg[:, 32:33], -1.0)
        nc.vector.memset(Tpos2[:, 32:33], 2.0)

        # per-(layer, group) ones-mask lhsT [128, 32]: col j = -1 if o is F
        ones_mask = {}
        for li in range(1, 8):
            O_l = LAYERS[li][0]
            flav = _flavors(li)
            for q in range(max(1, O_l // 32)):
                nQ_l = max(1, O_l // 32)
                cols = [j for j in range(32)
                        if q * 32 + j < O_l
                        and flav[(j * nQ_l + q) % 32] == "F"]
                if not cols:
                    continue
                m = persist.tile([128, 32], bf16, name=f"om{li}_{q}",
                                 tag=f"om{li}_{q}")
                nc.vector.memset(m, 0.0)
                for j in cols:
                    nc.vector.memset(m[:, j:j + 1], -1.0)
                ones_mask[(li, q)] = m

        # load input x -> Ap[0] interior (f32 -> bf16); rows = (b, c)
        a0v = Ap[0].rearrange("p (h w) -> p h w", h=34)
        with tc.tile_pool(name="xload", bufs=1) as xpool:
            xs = xpool.tile([48, 1024], f32, name="xs", tag="xs")
            nc.sync.dma_start(out=xs, in_=bass.AP(
                tensor=x_d, offset=0, ap=[[1024, 48], [1, 1024]]))
            nc.vector.tensor_copy(
                a0v[0:48, 1:33, 1:33],
                xs.rearrange("p (h w) -> p h w", h=32))

        # per-layer weights, f = blk*C + c:
        #   wpos[t] [rows, O] f32 = +w (custom-DVE ops), wneg = -w (ACT bias)
        wpos_all, wneg_all, negs_all, bb_all = [], [], [], []
        for i, (O, C, H, _) in enumerate(LAYERS):
            CKK = C * 9
            T = (CKK + 127) // 128
            wpos_l, wneg_l = [], []
            for t in range(T):
                rows = min(128, CKK - t * 128)
                wps = wpool.tile([rows, O], f32, name=f"wps{i}_{t}",
                                 tag="wstage", bufs=2)
                blk0 = t * 128 // C
                nblk = rows // C
                for bi in range(nblk):
                    blk = blk0 + bi
                    nc.sync.dma_start(
                        out=wps[bi * C:(bi + 1) * C, :],
                        in_=bass.AP(tensor=w_d[i], offset=blk,
                                    ap=[[9, C], [C * 9, O]]))
                wp = wpool.tile([rows, O], f32, name=f"wpos{i}_{t}",
                                tag=f"wpos{i}_{t}")
                nc.vector.tensor_copy(wp, wps)
                wn = wpool.tile([rows, O], f32, name=f"wneg{i}_{t}",
                                tag=f"wneg{i}_{t}")
                nc.vector.tensor_scalar_mul(wn, wps, -1.0)
                wpos_l.append(wp)
                wneg_l.append(wn)
            wpos_all.append(wpos_l)
            wneg_all.append(wneg_l)

            st = wpool.tile([O, 1], f32, name=f"st{i}", tag="st_tmp", bufs=2)
            nc.sync.dma_start(out=st, in_=bass.AP(tensor=s_d[i], offset=0,
                                                  ap=[[1, O], [1, 1]]))
            ns = wpool.tile([O, 1], f32, name=f"negs{i}", tag=f"negs{i}")
            nc.vector.tensor_scalar_mul(ns, st, -1.0)
            negs_all.append(ns)
            bb = wpool.tile([O, 1], f32, name=f"bb{i}", tag=f"bb{i}")
            nc.sync.dma_start(out=bb, in_=bass.AP(tensor=b_d[i], offset=0,
                                                  ap=[[1, O], [1, 1]]))
            bb_all.append(bb)

        # FC weights [c, (hw, cls)] bf16 and bias [10, 1] f32
        fcs = persist.tile([128, 160], f32, name="fcs", tag="fcs")
        nc.sync.dma_start(out=fcs, in_=bass.AP(
            tensor=fcw_d, offset=0, ap=[[16, 128], [1, 16], [2048, 10]]))
        fcw = persist.tile([128, 160], bf16, name="fcw", tag="fcw")
        nc.vector.tensor_copy(fcw, fcs)
        fcb = persist.tile([10, 1], f32, name="fcb", tag="fcb")
        nc.sync.dma_start(out=fcb, in_=bass.AP(tensor=fcb_d, offset=0,
                                               ap=[[1, 10], [1, 1]]))

        # --- L1 replicated-path constants ---
        # wneg1x4 [108, 32]: rows 27r+f = -w1[o, f] (4 replicas)
        wpos4 = persist.tile([108, 32], f32, name="wpos4", tag="wpos4")
        for r in range(4):
            for blk in range(9):
                nc.sync.dma_start(
                    out=wpos4[27 * r + 3 * blk:27 * r + 3 * blk + 3, :],
                    in_=bass.AP(tensor=w_d[0], offset=blk, ap=[[9, 3], [27, 32]]))
        wneg1x4 = persist.tile([108, 32], f32, name="wneg1x4", tag="wneg1x4")
        nc.vector.tensor_scalar_mul(wneg1x4, wpos4, -1.0)
        # R2p/R2n [108, 64] bf16: col 32+8r is +/-1 on replica-r rows.
        # Build via a [1, 256] onehot strip (partition-0 writes are aligned),
        # then broadcast-DMA each 64-segment to the replica's 27 rows.
        strip = persist.tile([1, 512], bf16, name="strip", tag="strip")
        nc.vector.memset(strip, 0.0)
        for r in range(4):
            nc.vector.memset(strip[0:1, 64 * r + 32 + 8 * r:64 * r + 33 + 8 * r], 1.0)
            nc.vector.memset(strip[0:1, 256 + 64 * r + 32 + 8 * r:
                                  256 + 64 * r + 33 + 8 * r], -1.0)
        strip_d = nc.dram_tensor("r2strip", [512], bf16)
        nc.sync.dma_start(out=bass.AP(tensor=strip_d, offset=0,
                                      ap=[[512, 1], [1, 512]]),
                          in_=strip[0:1, :])
        R2p = persist.tile([108, 64], bf16, name="R2p", tag="R2p")
        R2n = persist.tile([108, 64], bf16, name="R2n", tag="R2n")
        for r in range(4):
            nc.sync.dma_start(out=R2p[27 * r:27 * r + 27, :], in_=bass.AP(
                tensor=strip_d, offset=64 * r, ap=[[0, 27], [1, 64]]))
            nc.sync.dma_start(out=R2n[27 * r:27 * r + 27, :], in_=bass.AP(
                tensor=strip_d, offset=256 + 64 * r, ap=[[0, 27], [1, 64]]))
        # negs1e/bb1e per o-group g: [32,1], row 8r+c = value[8g+c]
        negs1e, bb1e = [], []
        for g in range(4):
            se = wpool.tile([32, 1], f32, name=f"se1_{g}", tag=f"se1_{g}")
            be = wpool.tile([32, 1], f32, name=f"be1_{g}", tag=f"be1_{g}")
            for r in range(4):
                nc.sync.dma_start(out=se[8 * r:8 * r + 8, :], in_=bass.AP(
                    tensor=s_d[0], offset=8 * g, ap=[[1, 8], [1, 1]]))
                nc.sync.dma_start(out=be[8 * r:8 * r + 8, :], in_=bass.AP(
                    tensor=b_d[0], offset=8 * g, ap=[[1, 8], [1, 1]]))
            ne = wpool.tile([32, 1], f32, name=f"ne1_{g}", tag=f"ne1_{g}")
            nc.vector.tensor_scalar_mul(ne, se, -1.0)
            negs1e.append(ne)
            bb1e.append(be)

        def l1_body():
            # layer 1, replicated: P1 [108, 1024] rows 27r+f, image b0+r
            O, C, H = 32, 3, 32
            W = H
            srcv = Ap[0].rearrange("p (h w) -> p h w", h=H + 2)
            dstv = Ap[1].rearrange("c (b h w) -> c b h w", b=BC, h=H + 2)
            with ExitStack() as lctx:
                p1pool = lctx.enter_context(tc.tile_pool(name="patchL1", bufs=2))
                for bch in range(4):
                    b0 = bch * 4
                    P1 = p1pool.tile([108, 1024], bf16, name=f"P1_{bch}", tag="P1")
                    for r in range(4):
                        for blk in range(9):
                            dh, dw = blk // 3, blk % 3
                            nc.sync.dma_start(
                                out=P1[27 * r + 3 * blk:27 * r + 3 * blk + 3,
                                       :].rearrange("c (h w) -> c h w", h=H),
                                in_=srcv[3 * (b0 + r):3 * (b0 + r) + 3,
                                         dh:dh + H, dw:dw + W])
                    for g in range(4):
                        ps = pspool.tile([32, 1024], f32, name=f"psL1_{bch}_{g}",
                                         tag="ps")
                        for ol in range(8):
                            o = 8 * g + ol
                            r1 = dpool.tile([108, 1024], bf16, name="r1L1", tag="d")
                            r2 = dpool.tile([108, 1024], bf16, name="r2L1", tag="d")
                            nc.vector.tensor_scalar(
                                r1, P1, wneg1x4[:, o:o + 1], 0.0, A.add, A.max)
                            nc.vector.tensor_scalar(
                                r2, P1, wneg1x4[:, o:o + 1], 0.0, A.add, A.min)
                            for sl in range(2):
                                nc.tensor.matmul(
                                    ps[0:32, sl * 512:(sl + 1) * 512],
                                    R2p[:, 32 - ol:64 - ol],
                                    r1[:, sl * 512:(sl + 1) * 512],
                                    start=(ol == 0), stop=False,
                                    tile_position=(0, 0), skip_group_check=True)
                                nc.tensor.matmul(
                                    ps[0:32, sl * 512:(sl + 1) * 512],
                                    R2n[:, 32 - ol:64 - ol],
                                    r2[:, sl * 512:(sl + 1) * 512],
                                    start=False, stop=(ol == 7),
                                    tile_position=(0, 0), skip_group_check=True)
                        tmpL1 = dpool.tile([32, 1024], bf16, name="tmpL1",
                                           tag="tmpl1", bufs=2)
                        nc.scalar.activation(tmpL1, ps, AF.Relu,
                                             bias=bb1e[g], scale=negs1e[g])
                        for r in range(4):
                            nc.sync.dma_start(
                                out=dstv[8 * g:8 * g + 8, b0 + r, 1:H + 1, 1:W + 1],
                                in_=tmpL1[8 * r:8 * r + 8, :].rearrange(
                                    "c (h w) -> c h w", h=H))

        def net_body():
            l1_body()
            for li, (O, C, H, pool_after) in enumerate(LAYERS):
                if li == 0:
                    continue
                CKK = C * 9
                T = (CKK + 127) // 128
                Hp = H + 2
                W = H
                src = Ap[li]
                srcv = src.rearrange("c (b h w) -> c b h w", b=BC, h=Hp)
                nQ = max(1, O // 32)
                # batch chunking: big layers processed in halves
                n_bch = 4 if H == 32 else (2 if H == 16 else 1)
                bcs = BC // n_bch
                npix_c = bcs * H * W
                # psum pix chunk (4 banks/tile x 2 bufs = all 8 banks)
                pch = min(npix_c, 2048)

                with ExitStack() as lctx:
                    ppool = lctx.enter_context(
                        tc.tile_pool(name=f"patch{li}", bufs=1))
                    tpool = (lctx.enter_context(
                        tc.tile_pool(name=f"ptmp{li}", bufs=1))
                        if pool_after else None)

                    for bch in range(n_bch):
                        b0 = bch * bcs
                        # --- build patch tiles via SBUF->SBUF DMA ---
                        pt = []
                        for t in range(T):
                            rows = min(128, CKK - t * 128)
                            p = ppool.tile([rows, npix_c], bf16,
                                           name=f"p{li}_{bch}_{t}", tag=f"pt{t}")
                            pt.append(p)
                        for blk in range(9):
                            dh, dw = blk // 3, blk % 3
                            gr = blk * C
                            t, r0 = gr // 128, gr % 128
                            if H == 8 and r0 % 32 == 0 and C % 32 == 0:
                                nc.gpsimd.tensor_copy(
                                    pt[t][r0:r0 + C, :].rearrange(
                                        "c (b h w) -> c b h w", b=bcs, h=H),
                                    srcv[0:C, b0:b0 + bcs, dh:dh + H, dw:dw + W])
                            else:
                                for bi in range(bcs):
                                    nc.sync.dma_start(
                                        out=pt[t][r0:r0 + C,
                                                  bi * H * W:(bi + 1) * H * W].rearrange(
                                            "c (h w) -> c h w", h=H),
                                        in_=srcv[0:C, b0 + bi, dh:dh + H, dw:dw + W])

                        if pool_after:
                            dest = tpool.tile([O, npix_c], bf16,
                                              name=f"tmp{li}_{bch}", tag="tmp")
                        # --- absdiff + PE reduce + epilogue, per psum chunk ---
                        flav = _flavors(li)
                        npair = T // 2
                        for p0 in range(0, npix_c, pch):
                            ps = pspool.tile([max(32, O), pch], f32,
                                             name=f"ps{li}_{bch}_{p0}", tag="ps")
                            nsl = pch // 512
            # ones-streams are emitted AFTER the o-streams (inside the
                            # same psum accumulation group) so the first PE
                            # work of a chunk is not gated on every patch DMA
                            for j in range(32):
                                for q in range(nQ):
                                    o = q * 32 + j
                                    if o >= O:
                                        continue
                                    fl = flav[(j * nQ + q) % 32]
                                    streams = []
                                    if fl == "A":
                                        for t in range(T):
                                            rows = pt[t].shape[0]
                                            d = dpool.tile([rows, pch], bf16,
                                                           name=f"d{li}",
                                                           tag="dA", bufs=7)
                                            nc.scalar.activation(
                                                d, pt[t][:, p0:p0 + pch], AF.Abs,
                                                bias=wneg_all[li][t][:, o:o + 1],
                                                scale=1.0)
                                            streams.append((rows, d, Tpos))
                                    elif fl == "F":
                                        for t in range(T):
                                            rows = pt[t].shape[0]
                                            d = dpool.tile([rows, pch], bf16,
                                                           name=f"df{li}", tag="d")
                                            nc.vector.tensor_scalar(
                                                d, pt[t][:, p0:p0 + pch],
                                                wneg_all[li][t][:, o:o + 1],
                                                0.0, A.add, A.max)
                                            streams.append((rows, d, Tpos2))
                                    else:
                                        for pi in range(npair):
                                            t0, t1 = 2 * pi, 2 * pi + 1
                                            rows = pt[t0].shape[0]
                                            d = dpool.tile([rows, pch], bf16,
                                                           name=f"dp{li}", tag="d")
                                            nc.vector._custom_dve(
                                                PAIRSAD, out=d[:, :],
                                                in0=pt[t0][:, p0:p0 + pch],
                                                in1=pt[t1][:, p0:p0 + pch],
                                                s0=wpos_all[li][t0][:, o:o + 1],
                                                s1=wpos_all[li][t1][:, o:o + 1])
                                            streams.append((rows, d, Tpos))
                                        if T % 2:
                                            t0 = T - 1
                                            rows = pt[t0].shape[0]
                                            d = dpool.tile([rows, pch], bf16,
                                                           name=f"ds{li}", tag="d")
                                            nc.vector._custom_dve(
                                                ABSD, out=d[:, :],
                                                in0=pt[t0][:, p0:p0 + pch],
                                                s0=wpos_all[li][t0][:, o:o + 1])
                                            streams.append((rows, d, Tpos))
                                    nstr = len(streams)
                                    last_o = ((j == 31) or (o == O - 1))
                                    for si, (rows, d, basis) in enumerate(streams):
                                        for sl in range(nsl):
                                            nc.tensor.matmul(
                                                ps[q * 32:q * 32 + 32,
                                                   sl * 512:(sl + 1) * 512],
                                                basis[0:rows, 32 - j:64 - j],
                                                d[:, sl * 512:(sl + 1) * 512],
                                                start=(j == 0 and si == 0),
                                                stop=(last_o and si == nstr - 1
                                                      and (li, q) not in ones_mask),
                                                tile_position=(0, 32 * q),
                                                skip_group_check=True)
                            # trailing ones-streams: add -sum(x) into F rows
                            for q in range(nQ):
                                om = ones_mask.get((li, q))
                                if om is None:
                                    continue
                                for t in range(T):
                                    rows = pt[t].shape[0]
                                    for sl in range(nsl):
                                        nc.tensor.matmul(
                                            ps[q * 32:q * 32 + 32,
                                               sl * 512:(sl + 1) * 512],
                                            om[0:rows, :],
                                            pt[t][:, p0 + sl * 512:
                                                  p0 + (sl + 1) * 512],
                                            start=False,
                                            stop=(t == T - 1 and sl == nsl - 1),
                                            tile_position=(0, 32 * q),
                                            skip_group_check=True)
                            # epilogue: relu(-s * psum + b)
                            if pool_after:
                                nc.scalar.activation(
                                    dest[:, p0:p0 + pch], ps[0:O, :], AF.Relu,
                                    bias=bb_all[li][:, :], scale=negs_all[li][:, :])
                            else:
                                Hn = H  # same spatial size, next layer pad Hn+2
                                dv = Ap[li + 1].rearrange(
                                    "c (b h w) -> c b h w", b=BC, h=Hn + 2)
                                # pixel range [p0, p0+pch) within this bchunk:
                                # whole images per chunk (pch % (H*W) == 0)
                                i0 = b0 + p0 // (H * W)
                                ni = pch // (H * W)
                                nc.scalar.activation(
                                    dv[0:O, i0:i0 + ni, 1:H + 1, 1:W + 1],
                                    ps[0:O, :].rearrange(
                                        "c (b h w) -> c b h w", b=ni, h=H),
                                    AF.Relu,
                                    bias=bb_all[li][:, :], scale=negs_all[li][:, :])

                        # --- maxpool 2x2 -> next padded tensor (or A8) ---
                        if pool_after:
                            dv4 = dest.rearrange("c (b h w) -> c b h w", b=bcs, h=H)
                            m1 = tpool.tile([O, npix_c // 2], bf16,
                                            name=f"m1_{li}_{bch}", tag="m1")
                            m1v = m1.rearrange("c (b h w) -> c b h w", b=bcs, h=H)
                            nc.vector.tensor_tensor(
                                m1v, dv4[:, :, :, 0::2], dv4[:, :, :, 1::2], A.max)
                            if li == 7:
                                nxt = A8.rearrange("c (b h w) -> c b h w",
                                                   b=BC, h=4)[0:O, b0:b0 + bcs]
                            else:
                                Hn = H // 2
                                nxt = Ap[li + 1].rearrange(
                                    "c (b h w) -> c b h w", b=BC, h=Hn + 2)[
                                    0:O, b0:b0 + bcs, 1:Hn + 1, 1:Hn + 1]
                            nc.vector.tensor_tensor(
                                nxt, m1v[:, :, 0::2, :], m1v[:, :, 1::2, :], A.max)

            # --- FC: out[cls, b] = sum_c,hw A8[c, b*16+hw] * fcw[c, hw*10+cls] ---
            psf = pspool.tile([32, 512], f32, name="psf", tag="ps")
            for hw in range(16):
                nc.tensor.matmul(
                    psf[0:10, 0:BC],
                    fcw[:, hw * 10:(hw + 1) * 10],
                    bass.AP(tensor=A8.tensor, offset=A8.offset + hw,
                            ap=[list(A8.ap[0]), [16, BC]]),
                    start=(hw == 0), stop=(hw == 15), skip_group_check=True)
            osb = persist.tile([10, BC], f32, name="osb", tag="osb")
            nc.scalar.activation(osb, psf[0:10, 0:BC], AF.Identity,
                                 bias=fcb[:, :], scale=1.0)
            nc.sync.dma_start(
                out=bass.AP(tensor=out_d, offset=0, ap=[[1, 10], [10, BC]]),
                in_=osb)

        if loop_k > 1:
            with tc.For_i(0, loop_k, 1):
                net_body()
        else:
            net_body()

    nc.compile()
    return nc


def _get_nc(cfg=None):
    key = str(sorted((cfg or {}).items()))
    if key not in _CACHE:
        _CACHE[key] = _build(cfg)
    return _CACHE[key]


def make_in_maps(inputs):
    full = dict(inputs)
    x = np.ascontiguousarray(full["x"], dtype=np.float32)
    reps = {}
    for i in range(1, 9):
        w = np.ascontiguousarray(full[f"w{i}"], np.float32)
        s = np.ascontiguousarray(full[f"s{i}"], np.float32)
        b = np.ascontiguousarray(full[f"b{i}"], np.float32)
        O = w.shape[0]
        flav = _flavors(i - 1)
        nQ_l = max(1, O // 32)
        is_f = np.array(
            [flav[((o % 32) * nQ_l + o // 32) % 32] == "F" for o in range(O)],
            np.float32)
        wsum = w.reshape(O, -1).sum(axis=1)
        reps[f"w{i}"] = w
        reps[f"s{i}"] = s
        reps[f"bbx{i}"] = (b - s * wsum * is_f).astype(np.float32)
    reps["fc_w"] = np.ascontiguousarray(full["fc_w"], np.float32)
    reps["fc_b"] = np.ascontiguousarray(full["fc_b"], np.float32)
    in_maps = []
    for c in range(N_CORES):
        m = {"x": x[c * BC:(c + 1) * BC]}
        m.update(reps)
        in_maps.append(m)
    return in_maps


def kernel(**inputs):
    from concourse import bass_utils
    nc = _get_nc()
    in_maps = make_in_maps(inputs)
    res = bass_utils.run_bass_kernel_spmd(nc, in_maps,
                                          core_ids=list(range(N_CORES)))
    return np.concatenate([r["out"] for r in res.results], axis=0)

